# revision 5
# baseline (speedup 1.0000x reference)
"""Trainium2 Bass kernel for nn_MultiHeadAttention_42125039239620.

Semantics (faithful to reference.py):
  qh/kh/vh = per-head projections of q,k,v            [B,H,S,hd]
  scores   = qh @ kh^T / 8; masked rows/cols -> 0; causal strict-upper -> -inf
  attn     = softmax(scores); O = attn @ vh           [B,H,S,hd]
  out      = RAW VIEW of O as [B,S,H*hd] (memory reinterpretation, no head
             transpose!) @ Wo.
  The raw view decomposes per head: out[b, 128h:128(h+1), :] =
      O[b,h].reshape(128, 16*hd) @ Wo[0]
  so each (b, h) owns 128 exclusive output rows -> the 8-core unshard is a
  pure concatenation (no inter-core reduction).

Sharding: core c -> batch c//4, heads 4*(c%4) .. 4*(c%4)+3.

Device pipeline per core (bf16 matmuls, fp32 PSUM accumulate):
  - host feeds qT ( (q*keep/8)^T ), kT ( (k*keep)^T ), vT (v^T) in bf16; the
    pad mask is folded into q/k so masked score entries become exactly 0
    (exp(0)=1, matching the reference's where(pad, 0, scores)).
  - projections produce qh^T/kh^T [64,S] per head (2 heads packed per matmul)
    and vh [t,hd] with an interleaved ones column (PV matmul then yields
    softmax denominators for free as psum row 64).
  - scores^T chunks [128t, 512s] = matmul(lhsT=kh^T chunk, rhs=qh^T chunk);
    exp on ScalarE (no max subtraction: scores are bounded, |s| < ~30);
    causal handled by only computing t<=s chunks + 4 static triangular 0/1
    mask multiplies on the diagonal blocks.
  - normalization: recip of sums (row 64) broadcast across partitions via
    GpSimd, fused into the PSUM->SBUF copy of O^T, written in the scrambled
    layout the Wo stage needs.
  - Wo stage: out rows 128hl..128hl+127 = sum_c O_scr[:,128c:128c+128]^T @
    Wo[64c:64c+64, :].
"""

import sys

sys.path.insert(0, "/opt/trn_rl_repo")

import numpy as np
import ml_dtypes

import concourse.bacc as bacc
import concourse.tile as tile
import concourse.mybir as mybir
from concourse.bass_utils import run_bass_kernel_spmd

BF16 = ml_dtypes.bfloat16
FP32 = mybir.dt.float32
BF = mybir.dt.bfloat16

B, S, D = 2, 2048, 1024
H, HD = 16, 64
NC = 8          # cores
HL = 4          # heads per core
SC = 512        # s-chunk width (matmul free dim)
NJ = S // SC    # 4 s-chunks
TC = 128        # t-chunk width (psum partition dim)
NTC = S // TC   # 16 t-chunks
DC = D // 128   # 8 d-chunks

_PROGRAM = None


def _build_program():
    nc = bacc.Bacc("TRN2", target_bir_lowering=False, debug=False, num_devices=NC)

    qT = nc.dram_tensor("qT", [D, S], BF, kind="ExternalInput")
    kT = nc.dram_tensor("kT", [D, S], BF, kind="ExternalInput")
    vT = nc.dram_tensor("vT", [D, S], BF, kind="ExternalInput")
    wq = nc.dram_tensor("wq", [D, HL * HD], BF, kind="ExternalInput")
    wk = nc.dram_tensor("wk", [D, HL * HD], BF, kind="ExternalInput")
    wv = nc.dram_tensor("wv", [D, HL * HD], BF, kind="ExternalInput")
    wo = nc.dram_tensor("wo", [D, D], BF, kind="ExternalInput")
    tri = nc.dram_tensor("tri", [4, TC, SC], BF, kind="ExternalInput")
    out = nc.dram_tensor("out", [HL * TC, D], mybir.dt.float32, kind="ExternalOutput")

    with tile.TileContext(nc) as tc:
        with (
            tc.tile_pool(name="big", bufs=1) as big,
            tc.tile_pool(name="acts", bufs=1) as acts,
            tc.tile_pool(name="exp", bufs=3) as expp,
            tc.tile_pool(name="small", bufs=2) as small,
            tc.tile_pool(name="ostage", bufs=2) as ostage,
            tc.tile_pool(name="ps_mm", bufs=2, space="PSUM") as ps_mm,
            tc.tile_pool(name="ps_sc", bufs=2, space="PSUM") as ps_sc,
            tc.tile_pool(name="ps_out", bufs=2, space="PSUM") as ps_out,
        ):
            # ---- load inputs ------------------------------------------------
            qT_sb = big.tile([128, DC, S], BF, tag="qT")
            nc.sync.dma_start(qT_sb[:], qT.rearrange("(c p) s -> p c s", p=128))
            kT_sb = big.tile([128, DC, S], BF, tag="kT")
            nc.sync.dma_start(kT_sb[:], kT.rearrange("(c p) s -> p c s", p=128))
            vT_sb = big.tile([128, DC, S], BF, tag="vT")
            nc.sync.dma_start(vT_sb[:], vT.rearrange("(c p) s -> p c s", p=128))
            wq_sb = big.tile([128, DC, HL * HD], BF, tag="wq")
            nc.sync.dma_start(wq_sb[:], wq.rearrange("(c p) n -> p c n", p=128))
            wk_sb = big.tile([128, DC, HL * HD], BF, tag="wk")
            nc.sync.dma_start(wk_sb[:], wk.rearrange("(c p) n -> p c n", p=128))
            wv_sb = big.tile([128, DC, HL * HD], BF, tag="wv")
            nc.sync.dma_start(wv_sb[:], wv.rearrange("(c p) n -> p c n", p=128))
            wo_sb = big.tile([64, 16, D], BF, tag="wo")
            nc.sync.dma_start(wo_sb[:], wo.rearrange("(c p) n -> p c n", p=64))
            tri_sb = big.tile([128, 4, SC], BF, tag="tri")
            nc.sync.dma_start(tri_sb[:], tri.rearrange("m p s -> p m s"))

            # ---- projections ------------------------------------------------
            # qh^T / kh^T: [128 (= head pair, 2x64), S] bf16, per pair.
            qh_sb = acts.tile([128, 2, S], BF, tag="qh")
            kh_sb = acts.tile([128, 2, S], BF, tag="kh")
            for (w_sb, dst) in ((wq_sb, qh_sb), (wk_sb, kh_sb)):
                src_in = qT_sb if dst is qh_sb else kT_sb
                for p in range(2):
                    for j in range(NJ):
                        ps = ps_mm.tile([128, SC], FP32, tag="mm")
                        for dc in range(DC):
                            nc.tensor.matmul(
                                ps[:],
                                w_sb[:, dc, 128 * p : 128 * (p + 1)],
                                src_in[:, dc, SC * j : SC * (j + 1)],
                                start=(dc == 0),
                                stop=(dc == DC - 1),
                            )
                        nc.vector.tensor_copy(
                            dst[:, p, SC * j : SC * (j + 1)], ps[:]
                        )

            # vh with interleaved ones column: [128 t, 16 tc, 4*65] bf16.
            vhp_sb = acts.tile([128, NTC, HL * 65], BF, tag="vhp")
            for t in range(NTC):
                ps = ps_mm.tile([128, HL * HD], FP32, tag="mm")
                for dc in range(DC):
                    nc.tensor.matmul(
                        ps[:],
                        vT_sb[:, dc, TC * t : TC * (t + 1)],
                        wv_sb[:, dc, :],
                        start=(dc == 0),
                        stop=(dc == DC - 1),
                    )
                nc.vector.tensor_copy(
                    vhp_sb[:, t, :].rearrange("p (h w) -> p h w", w=65)[:, :, 0:64],
                    ps[:].rearrange("p (h w) -> p h w", w=64),
                )
                nc.vector.memset(
                    vhp_sb[:, t, :].rearrange("p (h w) -> p h w", w=65)[:, :, 64:65],
                    1.0,
                )

            # ---- attention + output projection, per local head --------------
            oh_sb = acts.tile([64, HL, S], BF, tag="oh")  # scrambled O^T
            for hl in range(HL):
                p, off = hl // 2, 64 * (hl % 2)
                for j in range(NJ):
                    ntc = 4 * (j + 1)  # causal: t-chunks 0..ntc-1
                    o_ps = ps_out.tile([65, SC], FP32, tag="o")
                    for u in range(0, ntc, 2):
                        sc_ps = ps_sc.tile([128, 2 * SC], FP32, tag="sc")
                        for idx in range(2):
                            t = u + idx
                            nc.tensor.matmul(
                                sc_ps[:, SC * idx : SC * (idx + 1)],
                                kh_sb[off : off + 64, p, TC * t : TC * (t + 1)],
                                qh_sb[off : off + 64, p, SC * j : SC * (j + 1)],
                                start=True,
                                stop=True,
                            )
                        e_sb = expp.tile([128, 2 * SC], BF, tag="e")
                        nc.scalar.activation(
                            e_sb[:], sc_ps[:], mybir.ActivationFunctionType.Exp
                        )
                        for idx in range(2):
                            t = u + idx
                            m = t - 4 * j
                            if m >= 0:  # diagonal block: triangular mask
                                nc.vector.tensor_mul(
                                    e_sb[:, SC * idx : SC * (idx + 1)],
                                    e_sb[:, SC * idx : SC * (idx + 1)],
                                    tri_sb[:, m, :],
                                )
                        for idx in range(2):
                            t = u + idx
                            nc.tensor.matmul(
                                o_ps[:],
                                vhp_sb[:, t, 65 * hl : 65 * hl + 65],
                                e_sb[:, SC * idx : SC * (idx + 1)],
                                start=(t == 0),
                                stop=(t == ntc - 1),
                                skip_group_check=True,
                            )
                    # normalize: recip(sums) broadcast over partitions, fused
                    # into the PSUM->SBUF copy, written in scrambled layout.
                    sums_sb = small.tile([1, SC], FP32, tag="sums")
                    nc.vector.tensor_copy(sums_sb[:], o_ps[64:65, :])
                    rec_sb = small.tile([1, SC], FP32, tag="rec")
                    nc.vector.reciprocal_approx_fast(rec_sb[:], sums_sb[:])
                    bc_sb = small.tile([64, SC], FP32, tag="bc")
                    nc.gpsimd.partition_broadcast(bc_sb[:], rec_sb[:], channels=64)
                    # O^T[e, s] * rec[s]; s = 512j + 16r + c  ->  scrambled
                    # column 128c + 32j + r of oh_sb plane hl.
                    src = o_ps[0:64, :].rearrange("p (r c) -> p c r", c=16)
                    bcr = bc_sb[:].rearrange("p (r c) -> p c r", c=16)
                    dst = (
                        oh_sb[:, hl, :]
                        .rearrange("p (c rr) -> p c rr", rr=128)[
                            :, :, 32 * j : 32 * (j + 1)
                        ]
                    )
                    nc.vector.tensor_mul(dst, src, bcr)

                # Wo stage for this head: rows 128hl..128hl+127 of out.
                for n in range(2):
                    f_ps = ps_mm.tile([128, SC], FP32, tag="mm")
                    for c in range(16):
                        nc.tensor.matmul(
                            f_ps[:],
                            oh_sb[:, hl, 128 * c : 128 * (c + 1)],
                            wo_sb[:, c, SC * n : SC * (n + 1)],
                            start=(c == 0),
                            stop=(c == 15),
                        )
                    o_out = ostage.tile([128, SC], mybir.dt.float32, tag="ostage")
                    nc.vector.tensor_copy(o_out[:], f_ps[:])
                    nc.sync.dma_start(
                        out[TC * hl : TC * (hl + 1), SC * n : SC * (n + 1)], o_out[:]
                    )

    nc.compile()
    return nc


def _prep_inputs(q, k, v, Wq, Wk, Wv, Wo, mask):
    q = np.asarray(q, np.float32)
    k = np.asarray(k, np.float32)
    v = np.asarray(v, np.float32)
    Wq = np.asarray(Wq, np.float32)
    Wk = np.asarray(Wk, np.float32)
    Wv = np.asarray(Wv, np.float32)
    Wo = np.asarray(Wo, np.float32)
    mask = np.asarray(mask)

    keep = 1.0 - mask.astype(np.float32)  # [B, S]
    qTs, kTs, vTs = [], [], []
    for b in range(B):
        qTs.append(
            np.ascontiguousarray((q[b] * keep[b][:, None] * 0.125).T).astype(BF16)
        )
        kTs.append(np.ascontiguousarray((k[b] * keep[b][:, None]).T).astype(BF16))
        vTs.append(np.ascontiguousarray(v[b].T).astype(BF16))

    wqs, wks, wvs = [], [], []
    for g in range(4):
        hs = slice(4 * g, 4 * g + 4)
        wqs.append(
            np.ascontiguousarray(np.transpose(Wq[0, hs], (1, 0, 2)).reshape(D, HL * HD)).astype(BF16)
        )
        wks.append(
            np.ascontiguousarray(np.transpose(Wk[0, hs], (1, 0, 2)).reshape(D, HL * HD)).astype(BF16)
        )
        wvs.append(
            np.ascontiguousarray(np.transpose(Wv[0, hs], (1, 0, 2)).reshape(D, HL * HD)).astype(BF16)
        )
    wo_bf = np.ascontiguousarray(Wo[0]).astype(BF16)

    t_idx = np.arange(TC)[:, None]
    s_idx = np.arange(SC)[None, :]
    tri = np.stack(
        [(128 * m + t_idx <= s_idx).astype(np.float32) for m in range(4)]
    ).astype(BF16)

    in_maps = []
    for c in range(NC):
        b, g = c // 4, c % 4
        in_maps.append(
            {
                "qT": qTs[b],
                "kT": kTs[b],
                "vT": vTs[b],
                "wq": wqs[g],
                "wk": wks[g],
                "wv": wvs[g],
                "wo": wo_bf,
                "tri": tri,
            }
        )
    return in_maps


def _run(in_maps, trace=False):
    global _PROGRAM
    if _PROGRAM is None:
        _PROGRAM = _build_program()
    return run_bass_kernel_spmd(_PROGRAM, in_maps, list(range(NC)), trace=trace)


def kernel(q, k, v, Wq, Wk, Wv, Wo, mask, _trace=False):
    in_maps = _prep_inputs(q, k, v, Wq, Wk, Wv, Wo, mask)
    res = _run(in_maps, trace=_trace)
    final = np.zeros((B, S, D), np.float32)
    for c in range(NC):
        b, g = c // 4, c % 4
        final[b, 512 * g : 512 * (g + 1), :] = res.results[c]["out"]
    if _trace:
        kernel._last_exec_time_ns = res.exec_time_ns
        kernel._last_trace = res.instructions_and_trace
    return final


# revision 9
# speedup vs baseline: 1.1304x; 1.1304x over previous
"""Trainium2 Bass kernel for nn_MultiHeadAttention_42125039239620.

Semantics (faithful to reference.py):
  qh/kh/vh = per-head projections of q,k,v            [B,H,S,hd]
  scores   = qh @ kh^T / 8; masked rows/cols -> 0; causal strict-upper -> -inf
  attn     = softmax(scores); O = attn @ vh           [B,H,S,hd]
  out      = RAW VIEW of O as [B,S,H*hd] (memory reinterpretation, no head
             transpose!) @ Wo.
  The raw view decomposes per head: out[b, 128h:128(h+1), :] =
      O[b,h].reshape(128, 16*hd) @ Wo[0]
  so each (b, h) owns 128 exclusive output rows -> the 8-core unshard is a
  pure concatenation (no inter-core reduction).

Sharding: core c -> batch c//4, heads 4*(c%4) .. 4*(c%4)+3.

Device pipeline per core (bf16 matmuls, fp32 PSUM accumulate):
  - host feeds qT ( (q*keep/8)^T ), kT ( (k*keep)^T ), vT (v^T) in bf16; the
    pad mask is folded into q/k so masked score entries become exactly 0
    (exp(0)=1, matching the reference's where(pad, 0, scores)).
  - projections produce qh^T/kh^T [64,S] per head (2 heads packed per matmul)
    and vh [t,hd] with an interleaved ones column (PV matmul then yields
    softmax denominators for free as psum row 64).
  - scores^T chunks [128t, 512s] = matmul(lhsT=kh^T chunk, rhs=qh^T chunk);
    exp on ScalarE (no max subtraction: scores are bounded, |s| < ~30);
    causal handled by only computing t<=s chunks + 4 static triangular 0/1
    mask multiplies on the diagonal blocks.
  - normalization: recip of sums (row 64) broadcast across partitions via
    GpSimd, fused into the PSUM->SBUF copy of O^T, written in the scrambled
    layout the Wo stage needs.
  - Wo stage: out rows 128hl..128hl+127 = sum_c O_scr[:,128c:128c+128]^T @
    Wo[64c:64c+64, :].
"""

import sys

sys.path.insert(0, "/opt/trn_rl_repo")

import numpy as np
import ml_dtypes

import concourse.bacc as bacc
import concourse.tile as tile
import concourse.mybir as mybir
from concourse.bass_utils import run_bass_kernel_spmd

BF16 = ml_dtypes.bfloat16
FP32 = mybir.dt.float32
BF = mybir.dt.bfloat16

B, S, D = 2, 2048, 1024
H, HD = 16, 64
NC = 8          # cores
HL = 4          # heads per core
SC = 512        # s-chunk width (matmul free dim)
NJ = S // SC    # 4 s-chunks
TC = 128        # t-chunk width (psum partition dim)
NTC = S // TC   # 16 t-chunks
DC = D // 128   # 8 d-chunks

_PROGRAM = None


def _build_program():
    nc = bacc.Bacc("TRN2", target_bir_lowering=False, debug=False, num_devices=NC)

    qT = nc.dram_tensor("qT", [D, S], BF, kind="ExternalInput")
    kT = nc.dram_tensor("kT", [D, S], BF, kind="ExternalInput")
    vT = nc.dram_tensor("vT", [D, S], BF, kind="ExternalInput")
    wq = nc.dram_tensor("wq", [D, HL * HD], BF, kind="ExternalInput")
    wk = nc.dram_tensor("wk", [D, HL * HD], BF, kind="ExternalInput")
    wv = nc.dram_tensor("wv", [D, HL * HD], BF, kind="ExternalInput")
    wo = nc.dram_tensor("wo", [D, D], BF, kind="ExternalInput")
    tri = nc.dram_tensor("tri", [4, TC, SC], BF, kind="ExternalInput")
    out = nc.dram_tensor("out", [HL * TC, D], mybir.dt.float32, kind="ExternalOutput")

    with tile.TileContext(nc) as tc:
        with (
            tc.tile_pool(name="big", bufs=1) as big,
            tc.tile_pool(name="acts", bufs=1) as acts,
            tc.tile_pool(name="exp", bufs=2) as expp,
            tc.tile_pool(name="small", bufs=2) as small,
            tc.tile_pool(name="ostage", bufs=2) as ostage,
            tc.tile_pool(name="ps_mm", bufs=2, space="PSUM") as ps_mm,
            tc.tile_pool(name="ps_sc", bufs=2, space="PSUM") as ps_sc,
            tc.tile_pool(name="ps_out", bufs=2, space="PSUM") as ps_out,
        ):
            # ---- load inputs ------------------------------------------------
            qT_sb = big.tile([128, DC, S], BF, tag="qT")
            nc.sync.dma_start(qT_sb[:], qT.rearrange("(c p) s -> p c s", p=128))
            kT_sb = big.tile([128, DC, S], BF, tag="kT")
            nc.sync.dma_start(kT_sb[:], kT.rearrange("(c p) s -> p c s", p=128))
            vT_sb = big.tile([128, DC, S], BF, tag="vT")
            nc.sync.dma_start(vT_sb[:], vT.rearrange("(c p) s -> p c s", p=128))
            wq_sb = big.tile([128, DC, HL * HD], BF, tag="wq")
            nc.sync.dma_start(wq_sb[:], wq.rearrange("(c p) n -> p c n", p=128))
            wk_sb = big.tile([128, DC, HL * HD], BF, tag="wk")
            nc.sync.dma_start(wk_sb[:], wk.rearrange("(c p) n -> p c n", p=128))
            wv_sb = big.tile([128, DC, HL * HD], BF, tag="wv")
            nc.sync.dma_start(wv_sb[:], wv.rearrange("(c p) n -> p c n", p=128))
            wo_sb = big.tile([128, DC, D], BF, tag="wo")
            nc.sync.dma_start(wo_sb[:], wo.rearrange("(c p) n -> p c n", p=128))
            tri_sb = big.tile([128, 4, SC], BF, tag="tri")
            nc.sync.dma_start(tri_sb[:], tri.rearrange("m p s -> p m s"))

            # ---- projections ------------------------------------------------
            # qh^T / kh^T: [128 (= head pair, 2x64), S] bf16, per pair.
            qh_sb = acts.tile([128, 2, S], BF, tag="qh")
            kh_sb = acts.tile([128, 2, S], BF, tag="kh")
            for (w_sb, dst) in ((wq_sb, qh_sb), (wk_sb, kh_sb)):
                src_in = qT_sb if dst is qh_sb else kT_sb
                for p in range(2):
                    for j in range(NJ):
                        ps = ps_mm.tile([128, SC], FP32, tag="mm")
                        for dc in range(DC):
                            nc.tensor.matmul(
                                ps[:],
                                w_sb[:, dc, 128 * p : 128 * (p + 1)],
                                src_in[:, dc, SC * j : SC * (j + 1)],
                                start=(dc == 0),
                                stop=(dc == DC - 1),
                            )
                        nc.vector.tensor_copy(
                            dst[:, p, SC * j : SC * (j + 1)], ps[:]
                        )

            # vh with interleaved ones column: [128 t, 16 tc, 4*65] bf16.
            vhp_sb = acts.tile([128, NTC, HL * 65], BF, tag="vhp")
            for t in range(NTC):
                ps = ps_mm.tile([128, HL * HD], FP32, tag="mm")
                for dc in range(DC):
                    nc.tensor.matmul(
                        ps[:],
                        vT_sb[:, dc, TC * t : TC * (t + 1)],
                        wv_sb[:, dc, :],
                        start=(dc == 0),
                        stop=(dc == DC - 1),
                    )
                nc.vector.tensor_copy(
                    vhp_sb[:, t, :].rearrange("p (h w) -> p h w", w=65)[:, :, 0:64],
                    ps[:].rearrange("p (h w) -> p h w", w=64),
                )
                nc.vector.memset(
                    vhp_sb[:, t, :].rearrange("p (h w) -> p h w", w=65)[:, :, 64:65],
                    1.0,
                )

            # ---- attention + output projection ------------------------------
            # Heads are processed in pairs: the even head of a pair lives in
            # SBUF partitions 0-63, the odd head in 64-127.  The two scores
            # matmuls of a pair (K=64 each) go to disjoint PE row groups and
            # run concurrently; one exp covers both heads.
            oh_sb = acts.tile([64, HL, S], BF, tag="oh")  # scrambled O^T
            oh2_sb = acts.tile([128, HL, S // 2], BF, tag="oh2")
            for p in range(2):
                for j in range(NJ):
                    ntc = 4 * (j + 1)  # causal: t-chunks 0..ntc-1
                    o_ps = [
                        ps_out.tile([65, SC], FP32, tag="o", name=f"o{par}")
                        for par in range(2)
                    ]
                    for t in range(ntc):
                        sc_ps = ps_sc.tile([128, 2 * SC], FP32, tag="sc")
                        for par in range(2):
                            off = 64 * par
                            nc.tensor.matmul(
                                sc_ps[:, SC * par : SC * (par + 1)],
                                kh_sb[off : off + 64, p, TC * t : TC * (t + 1)],
                                qh_sb[off : off + 64, p, SC * j : SC * (j + 1)],
                                start=True,
                                stop=True,
                            )
                        e_sb = expp.tile([128, 2 * SC], BF, tag="e")
                        nc.scalar.activation(
                            e_sb[:], sc_ps[:], mybir.ActivationFunctionType.Exp
                        )
                        m = t - 4 * j
                        if m >= 0:  # diagonal block: triangular mask
                            for par in range(2):
                                nc.vector.tensor_mul(
                                    e_sb[:, SC * par : SC * (par + 1)],
                                    e_sb[:, SC * par : SC * (par + 1)],
                                    tri_sb[:, m, :],
                                )
                        for par in range(2):
                            hl = 2 * p + par
                            nc.tensor.matmul(
                                o_ps[par][:],
                                vhp_sb[:, t, 65 * hl : 65 * hl + 65],
                                e_sb[:, SC * par : SC * (par + 1)],
                                start=(t == 0),
                                stop=(t == ntc - 1),
                                skip_group_check=True,
                            )
                    # normalize: recip(sums) broadcast over partitions, fused
                    # into the PSUM->SBUF copy, written in scrambled layout.
                    for par in range(2):
                        hl = 2 * p + par
                        sums_sb = small.tile([1, SC], FP32, tag="sums")
                        nc.vector.tensor_copy(sums_sb[:], o_ps[par][64:65, :])
                        rec_sb = small.tile([1, SC], FP32, tag="rec")
                        nc.vector.reciprocal_approx_fast(rec_sb[:], sums_sb[:])
                        bc_sb = small.tile([64, SC], FP32, tag="bc")
                        nc.gpsimd.partition_broadcast(
                            bc_sb[:], rec_sb[:], channels=64
                        )
                        # O^T[e, s] * rec[s]; s = 512j + 16r + c  ->  scrambled
                        # column 128c + 32j + r of oh_sb plane hl.
                        src = o_ps[par][0:64, :].rearrange("p (r c) -> p c r", c=16)
                        bcr = bc_sb[:].rearrange("p (r c) -> p c r", c=16)
                        dst = (
                            oh_sb[:, hl, :]
                            .rearrange("p (c rr) -> p c rr", rr=128)[
                                :, :, 32 * j : 32 * (j + 1)
                            ]
                        )
                        nc.vector.tensor_mul(dst, src, bcr)

                # Wo stage for the two heads of this pair.  First repack the
                # scrambled O^T so even c-blocks sit in partitions 0-63 and
                # odd c-blocks in 64-127 (SBUF->SBUF DMA): the Wo matmuls then
                # contract K=128 against Wo's natural 128-row chunks.
                for par in range(2):
                    hl = 2 * p + par
                    src3 = oh_sb[:, hl, :].rearrange(
                        "p (a two b) -> p two a b", two=2, b=128
                    )
                    dst3 = oh2_sb[:, hl, :].rearrange("p (a b) -> p a b", b=128)
                    nc.sync.dma_start(dst3[0:64], src3[:, 0])
                    nc.sync.dma_start(dst3[64:128], src3[:, 1])
                    f_ps = [
                        ps_mm.tile([128, SC], FP32, tag="mm", name=f"f{n}")
                        for n in range(2)
                    ]
                    for cc in range(8):
                        for n in range(2):
                            nc.tensor.matmul(
                                f_ps[n][:],
                                oh2_sb[:, hl, 128 * cc : 128 * (cc + 1)],
                                wo_sb[:, cc, SC * n : SC * (n + 1)],
                                start=(cc == 0),
                                stop=(cc == 7),
                                skip_group_check=True,
                            )
                    for n in range(2):
                        o_out = ostage.tile([128, SC], mybir.dt.float32, tag="os")
                        nc.vector.tensor_copy(o_out[:], f_ps[n][:])
                        nc.sync.dma_start(
                            out[TC * hl : TC * (hl + 1), SC * n : SC * (n + 1)],
                            o_out[:],
                        )

    nc.compile()
    return nc


def _prep_inputs(q, k, v, Wq, Wk, Wv, Wo, mask):
    q = np.asarray(q, np.float32)
    k = np.asarray(k, np.float32)
    v = np.asarray(v, np.float32)
    Wq = np.asarray(Wq, np.float32)
    Wk = np.asarray(Wk, np.float32)
    Wv = np.asarray(Wv, np.float32)
    Wo = np.asarray(Wo, np.float32)
    mask = np.asarray(mask)

    keep = 1.0 - mask.astype(np.float32)  # [B, S]
    qTs, kTs, vTs = [], [], []
    for b in range(B):
        qTs.append(
            np.ascontiguousarray((q[b] * keep[b][:, None] * 0.125).T).astype(BF16)
        )
        kTs.append(np.ascontiguousarray((k[b] * keep[b][:, None]).T).astype(BF16))
        vTs.append(np.ascontiguousarray(v[b].T).astype(BF16))

    wqs, wks, wvs = [], [], []
    for g in range(4):
        hs = slice(4 * g, 4 * g + 4)
        wqs.append(
            np.ascontiguousarray(np.transpose(Wq[0, hs], (1, 0, 2)).reshape(D, HL * HD)).astype(BF16)
        )
        wks.append(
            np.ascontiguousarray(np.transpose(Wk[0, hs], (1, 0, 2)).reshape(D, HL * HD)).astype(BF16)
        )
        wvs.append(
            np.ascontiguousarray(np.transpose(Wv[0, hs], (1, 0, 2)).reshape(D, HL * HD)).astype(BF16)
        )
    wo_bf = np.ascontiguousarray(Wo[0]).astype(BF16)

    t_idx = np.arange(TC)[:, None]
    s_idx = np.arange(SC)[None, :]
    tri = np.stack(
        [(128 * m + t_idx <= s_idx).astype(np.float32) for m in range(4)]
    ).astype(BF16)

    in_maps = []
    for c in range(NC):
        b, g = c // 4, c % 4
        in_maps.append(
            {
                "qT": qTs[b],
                "kT": kTs[b],
                "vT": vTs[b],
                "wq": wqs[g],
                "wk": wks[g],
                "wv": wvs[g],
                "wo": wo_bf,
                "tri": tri,
            }
        )
    return in_maps


def _run(in_maps, trace=False):
    global _PROGRAM
    if _PROGRAM is None:
        _PROGRAM = _build_program()
    return run_bass_kernel_spmd(_PROGRAM, in_maps, list(range(NC)), trace=trace)


def kernel(q, k, v, Wq, Wk, Wv, Wo, mask, _trace=False):
    in_maps = _prep_inputs(q, k, v, Wq, Wk, Wv, Wo, mask)
    res = _run(in_maps, trace=_trace)
    final = np.zeros((B, S, D), np.float32)
    for c in range(NC):
        b, g = c // 4, c % 4
        final[b, 512 * g : 512 * (g + 1), :] = res.results[c]["out"]
    if _trace:
        kernel._last_exec_time_ns = res.exec_time_ns
        kernel._last_trace = res.instructions_and_trace
    return final
